# revision 16
# baseline (speedup 1.0000x reference)
"""Causal self-attention (B=2, T=2048, C=1024, H=16) on 8 trn2 NeuronCores.

Sharding: batch x head-group. Core c handles batch b = c//4 and heads
[4*(c%4), 4*(c%4)+4). Each core computes qkv for its head slice, causal
attention, and a partial c_proj ([T, C] over its 256 input rows of W_proj);
the host sums the 4 partials per batch (data-parallel over b, tensor-parallel
over heads with the all-reduce done on host).

Device dataflow (per core):
  - qT, kT computed in [D', T] layout (D' = 256 local head dims), v in [T, D']
    layout, all from host-pre-transposed xT [C, T]; fp32r matmuls (every
    matmul operand tile is float32r-typed -- walrus requires producers of
    fp32r matmul inputs to declare fp32r output).
  - attention per head: S^T[k, q] = kT.T-slice @ qT-slice so that softmax's
    key dim is the PSUM partition dim; the padding mask becomes a
    per-partition bias of the exp activation.  exp(S/8 + maskNEG) -> U^T.
    Causal masking = one [128,128] upper-tri elementwise multiply per
    diagonal tile.  O^T[d, q] accumulates lhsT=v_aug [k,65] (65th col = 1.0
    gives the softmax denominator in row 64 for free), rhs=U^T.
  - normalize: denominator [1,1024] -> DMA-reshape to [64,16] -> reciprocal
    -> DMA broadcast [64,1024] -> multiply = yT [256, T], exactly the lhsT
    of c_proj.  partial = yT.T @ W_proj[256 rows].  Host adds b_proj.

Phase interleave (PE/ACT/DMA overlap, keeps the PE HAM clock warm):
  qkv(t<1024) -> attention(q<1024) -> qkv(t>=1024) -> proj(t<1024)
  -> attention(q>=1024) -> proj(t>=1024)
All PSUM lives in two pools: pool_x [128,1024] bufs=2 (S^T tiles, qkv
accumulators, proj accumulators share slots) + pool_o [65,1024] bufs=2.
"""

import contextlib
import functools
import sys

sys.path.insert(0, "/opt/trn_rl_repo")

import numpy as np

import concourse.bacc as bacc
import concourse.mybir as mybir
import concourse.tile as tile
from concourse import bass_utils
from concourse.alu_op_type import AluOpType

B, T, C, H, D = 2, 2048, 1024, 16, 64
NEG = -1e10
NCORES = 8
HEADS_PER_CORE = 4
DLOC = HEADS_PER_CORE * D  # 256 local head dims per core
F32 = mybir.dt.float32
F32R = mybir.dt.float32r
AF = mybir.ActivationFunctionType

NTB = T // 512  # 4 t-blocks in qkv phase
NKC = T // 128  # 16 k-chunks
NQB = 2  # attention q-blocks of 1024


def _r(ap):
    return ap.bitcast(F32R)


def _pieces(a, end=1024):
    """Split [a, end) at 512-boundaries (psum bank boundaries)."""
    cuts = [a]
    b = (a // 512 + 1) * 512
    while b < end:
        cuts.append(b)
        b += 512
    cuts.append(end)
    return list(zip(cuts[:-1], cuts[1:]))


class Ctx:
    pass


def _emit_qkv_tblock(nc, g, tb, with_weights=False):
    """qkv projections for t in [tb*512, (tb+1)*512).

    with_weights: interleave this tblock's x-chunk DMAs with the weight-chunk
    DMAs (first tblock only) so the first matmul chains start after ~400KB of
    DMA instead of after the full 7MB of inputs.
    """
    xts = [
        g.xpool.tile([128, 512], F32R, tag=f"xts{cc}", name=f"xts{cc}")
        for cc in range(8)
    ]
    for cc in range(8):
        nc.sync.dma_start(out=xts[cc], in_=g.xT_r[:, cc, tb * 512 : (tb + 1) * 512])
        if with_weights:
            for wsb, src in ((g.wq_sb, g.wq_src), (g.wk_sb, g.wk_src), (g.wv_sb, g.wv_src)):
                nc.sync.dma_start(out=wsb[cc], in_=src[:, cc, :])
    # qT / kT : [128 d', 512 t] chains packed in halves of a [128,1024] tile
    for wsb, bsb, dest in ((g.wq_sb, g.bq_sb, g.qT_sb), (g.wk_sb, g.bk_sb, g.kT_sb)):
        ps = g.pool_x.tile([128, 1024], F32, tag="px")
        for dt_ in range(2):
            for cc in range(8):
                nc.tensor.matmul(
                    ps[:, dt_ * 512 : (dt_ + 1) * 512],
                    wsb[cc][:, dt_ * 128 : (dt_ + 1) * 128],
                    xts[cc],
                    start=(cc == 0),
                    stop=(cc == 7),
                )
            nc.vector.tensor_scalar(
                out=dest[:, dt_, tb * 512 : (tb + 1) * 512],
                in0=ps[:, dt_ * 512 : (dt_ + 1) * 512],
                scalar1=bsb[:, dt_ : dt_ + 1],
                scalar2=None,
                op0=AluOpType.add,
            )
    # v: [128 t, 256 d'] chains, 4 t-subtiles packed in quarters
    psv = g.pool_x.tile([128, 1024], F32, tag="px")
    for ts in range(4):
        for cc in range(8):
            nc.tensor.matmul(
                psv[:, ts * 256 : (ts + 1) * 256],
                xts[cc][:, ts * 128 : (ts + 1) * 128],
                g.wv_sb[cc],
                start=(cc == 0),
                stop=(cc == 7),
            )
        kc = tb * 4 + ts
        for h in range(4):
            nc.vector.tensor_tensor(
                out=g.vaug[h][:, kc, 0:D],
                in0=psv[:, ts * 256 + h * D : ts * 256 + (h + 1) * D],
                in1=g.bvb_sb[:, h * D : (h + 1) * D],
                op=AluOpType.add,
            )


def _emit_attention_block(nc, g, h, m):
    """One head x one 1024-wide q-block of causal attention."""
    prow = (h % 2) * 64
    pi = h // 2
    njs = 8 * m + 8
    pso = g.pool_o.tile([D + 1, 1024], F32, tag="pso")
    last_bank0 = 8 * m + 3
    for j in range(njs):
        a = max(0, 128 * j - 1024 * m)
        pss = g.pool_x.tile([128, 1024], F32, tag="px")
        for c0, c1 in _pieces(a):
            nc.tensor.matmul(
                pss[:, c0:c1],
                g.kT_sb[prow : prow + 64, pi, j * 128 : (j + 1) * 128],
                g.qT_sb[prow : prow + 64, pi, m * 1024 + c0 : m * 1024 + c1],
                start=True,
                stop=True,
            )
        ut = g.utpool.tile([128, 1024], F32R, tag="ut")
        nc.scalar.activation(
            out=ut[:, a:1024],
            in_=pss[:, a:1024],
            func=AF.Exp,
            bias=g.mneg_sb[:, j : j + 1],
            scale=0.125,
        )
        if j >= 8 * m:
            nc.gpsimd.tensor_mul(ut[:, a : a + 128], ut[:, a : a + 128], g.tri_sb)
        for c0, c1 in _pieces(a):
            stop = j == (last_bank0 if c0 < 512 else njs - 1)
            nc.tensor.matmul(
                pso[:, c0:c1],
                g.vaug[h][:, j, :],
                ut[:, c0:c1],
                start=(j == 0),
                stop=stop,
            )
    # normalize: yT[h rows, m block] = O^T * (1/denom) broadcast.  The
    # denominator row is [1, 1024]; reciprocal there runs on one DVE lane
    # (6.5us), so DMA-reshape it to [64, 16] first.
    hm = h * NQB + m
    dn = g.rnpool.tile([1, 1024], F32, tag="dn")
    nc.vector.tensor_copy(dn, pso[D : D + 1, :])
    nc.sync.dma_start(out=g.rn_dram.ap()[hm : hm + 1, :], in_=dn)
    dn_rs = g.rnpool.tile([64, 16], F32, tag="dn_rs")
    nc.sync.dma_start(
        out=dn_rs, in_=g.rn_dram.ap()[hm, :].rearrange("(p f) -> p f", p=64)
    )
    rr = g.rnpool.tile([64, 16], F32, tag="rr")
    nc.vector.reciprocal(rr, dn_rs)
    nc.sync.dma_start(
        out=g.rn2_dram.ap()[hm, :].rearrange("(p f) -> p f", p=64), in_=rr
    )
    rnb = g.rnpool.tile([64, 1024], F32, tag="rnb")
    nc.sync.dma_start(
        out=rnb, in_=g.rn2_dram.ap()[hm : hm + 1, :].partition_broadcast(64)
    )
    nc.vector.tensor_tensor(
        out=g.yT_sb[prow : prow + 64, pi, m * 1024 : (m + 1) * 1024],
        in0=pso[0:D, :],
        in1=rnb,
        op=AluOpType.mult,
    )


def _emit_proj_tile(nc, g, i, out):
    # accumulate c'-chunk 1 (heads 2,3) first: attention emits heads in order
    # 2,3,0,1 so chunk 1's yT is ready early and chunk 0's normalize chain
    # hides behind the chunk-1 matmuls.
    psp = g.pool_x.tile([128, 1024], F32, tag="px")
    for step, ic in enumerate((1, 0)):
        for c0, c1 in _pieces(0):
            nc.tensor.matmul(
                psp[:, c0:c1],
                g.yT_sb[:, ic, i * 128 : (i + 1) * 128],
                g.wp_sb[:, ic, c0:c1],
                start=(step == 0),
                stop=(step == 1),
            )
    ob = g.outp.tile([128, C], F32, tag="ob")
    if i % 2 == 0:
        nc.vector.tensor_copy(ob, psp)
    else:
        nc.scalar.copy(ob, psp)
    nc.sync.dma_start(out=out.ap()[i * 128 : (i + 1) * 128, :], in_=ob)


def _build(ctx, nc, tc, ins, out, rn_dram, rn2_dram):
    g = Ctx()
    g.rn_dram, g.rn2_dram = rn_dram, rn2_dram

    singles = ctx.enter_context(tc.tile_pool(name="singles", bufs=1))

    # --- resident weights / constants (matmul inputs are F32R) ----------
    # per-chunk weight tiles: DMA'd interleaved with the first x chunks so
    # the first matmul chain starts after ~400KB of DMA, not 7MB
    g.wq_sb = [singles.tile([128, DLOC], F32R, name=f"wq{c}") for c in range(8)]
    g.wk_sb = [singles.tile([128, DLOC], F32R, name=f"wk{c}") for c in range(8)]
    g.wv_sb = [singles.tile([128, DLOC], F32R, name=f"wv{c}") for c in range(8)]
    g.wq_src = _r(ins["wq"].ap()).rearrange("(c p) m -> p c m", p=128)
    g.wk_src = _r(ins["wk"].ap()).rearrange("(c p) m -> p c m", p=128)
    g.wv_src = _r(ins["wv"].ap()).rearrange("(c p) m -> p c m", p=128)

    g.bq_sb = singles.tile([128, 2], F32, name="bq_sb")
    g.bk_sb = singles.tile([128, 2], F32, name="bk_sb")
    nc.sync.dma_start(out=g.bq_sb, in_=ins["bq"].ap().rearrange("i p -> p i"))
    nc.sync.dma_start(out=g.bk_sb, in_=ins["bk"].ap().rearrange("i p -> p i"))
    g.bvb_sb = singles.tile([128, DLOC], F32, name="bvb_sb")
    nc.sync.dma_start(out=g.bvb_sb, in_=ins["bv"].ap().partition_broadcast(128))
    g.mneg_sb = singles.tile([128, NKC], F32, name="mneg_sb")
    nc.sync.dma_start(out=g.mneg_sb, in_=ins["mneg"].ap())
    g.tri_sb = singles.tile([128, 128], F32, name="tri_sb")
    nc.sync.dma_start(out=g.tri_sb, in_=ins["tri"].ap())

    ones16 = singles.tile([128, NKC], F32, name="ones16")
    nc.vector.memset(ones16, 1.0)

    # --- persistent activations -----------------------------------------
    g.qT_sb = singles.tile([128, 2, T], F32R, tag="qT", name="qT_sb")
    g.kT_sb = singles.tile([128, 2, T], F32R, tag="kT", name="kT_sb")
    g.vaug = [
        singles.tile([128, NKC, D + 1], F32R, tag=f"vaug{h}", name=f"vaug{h}")
        for h in range(4)
    ]
    for h in range(4):
        nc.gpsimd.tensor_copy(g.vaug[h][:, :, D], ones16)
    g.yT_sb = singles.tile([128, 2, T], F32R, tag="yT", name="yT_sb")

    g.pool_x = ctx.enter_context(tc.tile_pool(name="pool_x", bufs=2, space="PSUM"))
    g.pool_o = ctx.enter_context(tc.tile_pool(name="pool_o", bufs=2, space="PSUM"))
    g.xpool = ctx.enter_context(tc.tile_pool(name="xpool", bufs=2))
    g.utpool = ctx.enter_context(tc.tile_pool(name="utpool", bufs=4))
    g.rnpool = ctx.enter_context(tc.tile_pool(name="rnpool", bufs=2))
    g.outp = ctx.enter_context(tc.tile_pool(name="outp", bufs=3))

    g.xT_r = _r(ins["xT"].ap()).rearrange("(c p) t -> p c t", p=128)

    # qkv for t < 1024
    _emit_qkv_tblock(nc, g, 0, with_weights=True)
    _emit_qkv_tblock(nc, g, 1)
    # attention for q < 1024 (needs only t < 1024 of q/k/v); heads 2,3 first
    # so proj's chunk-1-first accumulation can start before heads 0,1 finish
    for h in (2, 3, 0, 1):
        _emit_attention_block(nc, g, h, 0)
    # qkv for t >= 1024
    _emit_qkv_tblock(nc, g, 2)
    _emit_qkv_tblock(nc, g, 3)
    # c_proj weights (needed from proj phase on; DMA fits mid-kernel)
    g.wp_sb = singles.tile([128, 2, C], F32R, name="wp_sb")
    wp_src = _r(ins["wproj"].ap()).rearrange("(i p) n -> p i n", p=128)
    for ic in range(2):
        nc.sync.dma_start(out=g.wp_sb[:, ic, :], in_=wp_src[:, ic, :])
    # proj for t < 1024
    for i in range(8):
        _emit_proj_tile(nc, g, i, out)
    # attention for q >= 1024
    for h in (2, 3, 0, 1):
        _emit_attention_block(nc, g, h, 1)
    # proj for t >= 1024
    for i in range(8, 16):
        _emit_proj_tile(nc, g, i, out)


@functools.lru_cache(maxsize=1)
def _program():
    nc = bacc.Bacc("TRN2", target_bir_lowering=False, debug=False)
    shapes = {
        "xT": [C, T],
        "wq": [C, DLOC],
        "wk": [C, DLOC],
        "wv": [C, DLOC],
        "bq": [2, 128],
        "bk": [2, 128],
        "bv": [1, DLOC],
        "wproj": [DLOC, C],
        "mneg": [128, NKC],
        "tri": [128, 128],
    }
    ins = {
        name: nc.dram_tensor(name, shape, F32, kind="ExternalInput")
        for name, shape in shapes.items()
    }
    out = nc.dram_tensor("out", [T, C], F32, kind="ExternalOutput")
    rn_dram = nc.dram_tensor("rn_scratch", [8, 1024], F32, kind="Internal")
    rn2_dram = nc.dram_tensor("rn2_scratch", [8, 1024], F32, kind="Internal")
    with tile.TileContext(nc) as tc, contextlib.ExitStack() as ctx:
        _build(ctx, nc, tc, ins, out, rn_dram, rn2_dram)
    nc.compile()
    return nc


def make_in_maps(x, attention_mask, W_attn, b_attn, W_proj, b_proj):
    x = np.ascontiguousarray(np.asarray(x, dtype=np.float32))
    attention_mask = np.asarray(attention_mask, dtype=np.float32)
    W_attn = np.asarray(W_attn, dtype=np.float32)
    b_attn = np.asarray(b_attn, dtype=np.float32)
    W_proj = np.asarray(W_proj, dtype=np.float32)

    tri = (np.arange(128)[None, :] >= np.arange(128)[:, None]).astype(np.float32)
    in_maps = []
    for c in range(NCORES):
        b = c // 4
        g = c % 4
        cols = slice(g * DLOC, (g + 1) * DLOC)
        xT = np.ascontiguousarray(x[b].T)
        mneg = np.ascontiguousarray((attention_mask[b] * NEG).reshape(NKC, 128).T)
        in_maps.append(
            {
                "xT": xT,
                "wq": np.ascontiguousarray(W_attn[:, cols]),
                "wk": np.ascontiguousarray(W_attn[:, C : 2 * C][:, cols]),
                "wv": np.ascontiguousarray(W_attn[:, 2 * C : 3 * C][:, cols]),
                "bq": np.ascontiguousarray(b_attn[cols].reshape(2, 128)),
                "bk": np.ascontiguousarray(b_attn[C : 2 * C][cols].reshape(2, 128)),
                "bv": np.ascontiguousarray(b_attn[2 * C : 3 * C][cols].reshape(1, DLOC)),
                "wproj": np.ascontiguousarray(W_proj[g * DLOC : (g + 1) * DLOC, :]),
                "mneg": mneg,
                "tri": tri,
            }
        )
    return in_maps


def kernel(x, attention_mask, W_attn, b_attn, W_proj, b_proj, _res_hook=None):
    in_maps = make_in_maps(x, attention_mask, W_attn, b_attn, W_proj, b_proj)
    nc = _program()
    res = bass_utils.run_bass_kernel_spmd(nc, in_maps, core_ids=list(range(NCORES)))
    if _res_hook is not None:
        _res_hook(res)
    b_proj = np.asarray(b_proj, dtype=np.float32)
    y = np.zeros((B, T, C), dtype=np.float32)
    for c in range(NCORES):
        y[c // 4] += res.results[c]["out"]
    y += b_proj[None, None, :]
    return y
